# revision 16
# baseline (speedup 1.0000x reference)
"""Multi-head attention block (B=8, S=1024, D=1024, H=16) on 8 TRN2 NeuronCores.

Data-parallel over batch: core i computes batch element i end-to-end.
Per-core dataflow (bf16 compute, f32 PSUM accumulation; x/W pre-cast to
bf16 on the host):
  xT = transpose(x)                      (PE transposes)
  qkT[n,s] = W_qkv[:, :2048]^T @ x^T     (q rows pre-scaled by hd^-0.5)
  v[s,n]   = x @ W_qkv[:, 2048:]         (stored head-interleaved with a
                                          ones column per head -> "va", M=65)
  per head: scoresT[kj,qi] = kT^T q      (2 heads row-packed, K=64 each)
            expT = exp(scoresT)          (ScalarE, PSUM -> SBUF bf16)
            outT[c,qi], Z[qi] = va^T @ expT   (ones column accumulates Z)
            outT /= Z                    (approx reciprocal + partition bcast)
  out = outT^T @ W_proj + b_proj
QKV matmuls are interleaved pair-by-pair with attention so the PE never
idles while ScalarE works through the exps (keeps the HAM clock warm).
"""

import sys

if "/opt/trn_rl_repo" not in sys.path:
    sys.path.insert(0, "/opt/trn_rl_repo")

import ml_dtypes
import numpy as np

P = 128
S = 1024
D = 1024
H = 16
HD = 64
N_CORES = 8
SCALE = HD ** -0.5
ST = S // P   # 8 s-tiles
DT = D // P   # 8 d-tiles (contraction tiles)

_CACHE = {}


def _build():
    if "nc" in _CACHE:
        return _CACHE["nc"]

    from contextlib import ExitStack

    import concourse.bass as bass  # noqa: F401
    import concourse.mybir as mybir
    import concourse.tile as tile
    from concourse import bacc
    F32 = mybir.dt.float32
    BF = mybir.dt.bfloat16
    AluOp = mybir.AluOpType
    Act = mybir.ActivationFunctionType

    nc = bacc.Bacc(
        "TRN2", target_bir_lowering=False, debug=False, num_devices=N_CORES
    )

    x_d = nc.dram_tensor("x", [D, S], BF, kind="ExternalInput")  # x^T
    wqkv_d = nc.dram_tensor("W_qkv", [D, 3 * D], BF, kind="ExternalInput")
    bqkv_d = nc.dram_tensor("b_qkv", [3 * D], F32, kind="ExternalInput")
    wproj_d = nc.dram_tensor("W_proj", [D, D], BF, kind="ExternalInput")
    bproj_d = nc.dram_tensor("b_proj", [D], F32, kind="ExternalInput")
    out_d = nc.dram_tensor("out", [S, D], F32, kind="ExternalOutput")

    with tile.TileContext(nc) as tc, ExitStack() as ctx:
        const = ctx.enter_context(tc.tile_pool(name="const", bufs=1))
        persist = ctx.enter_context(tc.tile_pool(name="persist", bufs=1))
        # PSUM: "big" [128,1024]f32 tiles (2 banks) x3 + "pso" [65,512] x2
        psum = ctx.enter_context(tc.tile_pool(name="psum", bufs=3, space="PSUM"))
        psmall = ctx.enter_context(tc.tile_pool(name="psmall", bufs=2, space="PSUM"))
        small = ctx.enter_context(tc.tile_pool(name="small", bufs=2))

        # ---- constants ----
        ones_row = const.tile([1, P], BF)  # K=1 lhsT for rank-1 bias updates
        nc.gpsimd.memset(ones_row[:], 1.0)
        zbias = const.tile([P, 1], F32)  # zero bias for activation(Exp)
        nc.gpsimd.memset(zbias[:], 0.0)

        # b_qkv as per-partition columns for the first 2048 (q,k) outputs
        bqcol = const.tile([P, 16], F32)
        for nt in range(16):
            nc.sync.dma_start(
                bqcol[:, nt : nt + 1],
                bqkv_d[nt * P : (nt + 1) * P].rearrange("(p o) -> p o", o=1),
            )
        # b_qkv v-part and b_proj as bf16 rows (rank-1 matmul rhs)
        bvf = const.tile([1, D], F32, tag="brow_f")
        nc.sync.dma_start(bvf[:], bqkv_d[2 * D : 3 * D].rearrange("(o n) -> o n", o=1))
        bv_row = const.tile([1, D], BF)
        nc.vector.tensor_copy(bv_row[:], bvf[:])
        bpf = const.tile([1, D], F32, tag="brow_f")
        nc.sync.dma_start(bpf[:], bproj_d[:].rearrange("(o n) -> o n", o=1))
        bp_row = const.tile([1, D], BF)
        nc.vector.tensor_copy(bp_row[:], bpf[:])

        # ---- persistent tensors ----
        # qkT: only 2 pairs live at a time -> 4 rotating slots
        qk_pool = ctx.enter_context(tc.tile_pool(name="qk", bufs=4))
        va = [persist.tile([P, H * (HD + 1)], BF, name=f"va{s}") for s in range(ST)]
        outT = [persist.tile([P, S], BF, name=f"outT{t}") for t in range(DT)]
        xT = [persist.tile([P, S], BF, name=f"xT{t}") for t in range(DT)]
        Wqk = [persist.tile([P, 2 * D], BF, name=f"Wqk{t}") for t in range(DT)]

        for s8 in range(ST):
            nc.gpsimd.memset(va[s8][:], 1.0)  # ones columns survive the v copies

        # ---- DMA loads (bf16, pre-cast + pre-transposed x on host) ----
        for dt2 in range(DT):
            nc.sync.dma_start(xT[dt2][:], x_d[dt2 * P : (dt2 + 1) * P, :])
        for dt2 in range(DT):
            nc.sync.dma_start(Wqk[dt2][:], wqkv_d[dt2 * P : (dt2 + 1) * P, : 2 * D])


        qkT = {}

        def qkv_pair(hp):
            """qkT tiles for pair hp: q (scaled) and k, 2 rotating slots."""
            qt = qk_pool.tile([P, S], BF, name=f"q{hp}", tag="qk")
            kt = qk_pool.tile([P, S], BF, name=f"k{hp}", tag="qk")
            qkT[hp] = (qt, kt)
            for nt, dst in ((hp, qt), (8 + hp, kt)):
                ps = psum.tile([P, S], F32, name="ps_qk", tag="big")
                for dt2 in range(DT):
                    for sh in range(2):
                        nc.tensor.matmul(
                            ps[:, sh * 512 : (sh + 1) * 512],
                            Wqk[dt2][:, nt * P : (nt + 1) * P],
                            xT[dt2][:, sh * 512 : (sh + 1) * 512],
                            start=(dt2 == 0),
                            stop=(dt2 == DT - 1),
                        )
                if nt < 8:  # q: (psum + b) * scale
                    nc.vector.tensor_scalar(
                        dst[:], ps[:], bqcol[:, nt : nt + 1], SCALE,
                        AluOp.add, AluOp.mult,
                    )
                else:  # k: psum + b
                    nc.vector.tensor_scalar_add(dst[:], ps[:], bqcol[:, nt : nt + 1])

        def v_phase(xv_pool):
            Wv = [xv_pool.tile([P, D], BF, name=f"Wv{t}") for t in range(DT)]
            for dt2 in range(DT):
                nc.sync.dma_start(
                    Wv[dt2][:], wqkv_d[dt2 * P : (dt2 + 1) * P, 2 * D :]
                )
            # v = x @ Wv + bv, head-interleaved into va
            for s8 in range(ST):
                ps = psum.tile([P, S], F32, name="ps_v", tag="big")
                for dt2 in range(DT):
                    for sh in range(2):
                        nc.tensor.matmul(
                            ps[:, sh * 512 : (sh + 1) * 512],
                            xT[dt2][:, s8 * P : (s8 + 1) * P],
                            Wv[dt2][:, sh * 512 : (sh + 1) * 512],
                            start=(dt2 == 0),
                            stop=False,
                        )
                for sh in range(2):
                    nc.tensor.matmul(
                        ps[:, sh * 512 : (sh + 1) * 512],
                        ones_row[:],
                        bv_row[:, sh * 512 : (sh + 1) * 512],
                        start=False,
                        stop=True,
                    )
                nc.vector.tensor_copy(
                    va[s8][:].rearrange("p (h c) -> p h c", c=HD + 1)[:, :, 0:HD],
                    ps[:].rearrange("p (h c) -> p h c", c=HD),
                )

        exp_tiles = {}

        def scores_pair(hp, exp_pool):
            """scoresT + exp for heads (2hp, 2hp+1); fills exp_tiles[hp]."""
            expA = exp_pool.tile([P, ST * S], BF, name="expA", tag="expA")
            expB = exp_pool.tile([P, ST * S], BF, name="expB", tag="expB")
            exp_tiles[hp] = (expA, expB)
            qtile, ktile = qkT[hp]
            for jt in range(ST):
                psA = psum.tile([P, S], F32, name="psA", tag="big")
                psB = psum.tile([P, S], F32, name="psB", tag="big")
                for sh in range(2):
                    nc.tensor.matmul(
                        psA[:, sh * 512 : (sh + 1) * 512],
                        ktile[0:64, jt * P : (jt + 1) * P],
                        qtile[0:64, sh * 512 : (sh + 1) * 512],
                    )
                for sh in range(2):
                    nc.tensor.matmul(
                        psB[:, sh * 512 : (sh + 1) * 512],
                        ktile[64:128, jt * P : (jt + 1) * P],
                        qtile[64:128, sh * 512 : (sh + 1) * 512],
                        tile_position=(64, 0),
                    )
                nc.scalar.activation(
                    expA[:, jt * S : (jt + 1) * S], psA[:], Act.Exp, bias=zbias[:]
                )
                nc.scalar.activation(
                    expB[:, jt * S : (jt + 1) * S], psB[:], Act.Exp, bias=zbias[:]
                )

        def attnv_pair(hp):
            expA, expB = exp_tiles.pop(hp)
            for qh in range(2):
                for (ex, head) in ((expA, 2 * hp), (expB, 2 * hp + 1)):
                    pso = psmall.tile([HD + 1, 512], F32, name="pso", tag="pso")
                    for jt in range(ST):
                        nc.tensor.matmul(
                            pso[:],
                            va[jt][:, head * 65 : head * 65 + 65],
                            ex[:, jt * S + qh * 512 : jt * S + qh * 512 + 512],
                            start=(jt == 0),
                            stop=(jt == ST - 1),
                        )
                    po = (head % 2) * 64
                    reg = outT[hp][po : po + 64, qh * 512 : (qh + 1) * 512]
                    # copy unnormalized rows + Z out fast to release PSUM
                    nc.vector.tensor_copy(reg, pso[0:64, :])
                    zs = small.tile([1, 512], F32, name="zs", tag="zs")
                    nc.vector.tensor_copy(zs[:], pso[64:65, :])
                    rz = small.tile([1, 512], F32, name="rz", tag="rz")
                    nc.vector.reciprocal_approx_fast(out=rz[:], in_=zs[:])
                    bz = small.tile([P, 512], F32, name="bz", tag="bz")
                    nc.gpsimd.partition_broadcast(bz[:], rz[:])
                    nc.vector.tensor_mul(reg, reg, bz[po : po + 64, :])

        # ---- software-pipelined schedule ----
        with tc.tile_pool(name="xv", bufs=1) as xv_pool, \
             tc.tile_pool(name="exp", bufs=2) as exp_pool:
            v_phase(xv_pool)
            qkv_pair(0)
            qkv_pair(1)
            scores_pair(0, exp_pool)
            for hp in range(1, 8):
                if hp + 1 < 8:
                    qkv_pair(hp + 1)
                scores_pair(hp, exp_pool)
                attnv_pair(hp - 1)
            attnv_pair(7)

        # ---- output projection ----
        ob_pool = ctx.enter_context(tc.tile_pool(name="obp", bufs=2))
        wp_pool = ctx.enter_context(tc.tile_pool(name="wpp", bufs=1))
        Wp = [wp_pool.tile([P, D], BF, name=f"Wp{t}") for t in range(DT)]
        for dt2 in range(DT):
            nc.sync.dma_start(Wp[dt2][:], wproj_d[dt2 * P : (dt2 + 1) * P, :])
        for st in range(ST):
            ps = psum.tile([P, S], F32, name="ps_p", tag="big")
            for kt in range(DT):
                for sh in range(2):
                    nc.tensor.matmul(
                        ps[:, sh * 512 : (sh + 1) * 512],
                        outT[kt][:, st * P : (st + 1) * P],
                        Wp[kt][:, sh * 512 : (sh + 1) * 512],
                        start=(kt == 0),
                        stop=False,
                    )
            for sh in range(2):
                nc.tensor.matmul(
                    ps[:, sh * 512 : (sh + 1) * 512],
                    ones_row[:],
                    bp_row[:, sh * 512 : (sh + 1) * 512],
                    start=False,
                    stop=True,
                )
            ob = ob_pool.tile([P, S], F32, name="ob", tag="ob")
            for sh in range(2):
                nc.vector.tensor_copy(
                    ob[:, sh * 512 : (sh + 1) * 512], ps[:, sh * 512 : (sh + 1) * 512]
                )
                nc.sync.dma_start(
                    out_d[st * P : (st + 1) * P, sh * 512 : (sh + 1) * 512],
                    ob[:, sh * 512 : (sh + 1) * 512],
                )

    nc.compile()
    _CACHE["nc"] = nc
    return nc


def kernel(x, W_qkv, b_qkv, W_proj, b_proj, _trace=False):
    nc = _build()
    from concourse.bass_utils import run_bass_kernel_spmd

    bf = ml_dtypes.bfloat16
    wq = np.ascontiguousarray(W_qkv, dtype=np.float32).astype(bf)
    wp = np.ascontiguousarray(W_proj, dtype=np.float32).astype(bf)
    bq = np.ascontiguousarray(b_qkv, dtype=np.float32)
    bp = np.ascontiguousarray(b_proj, dtype=np.float32)
    in_maps = []
    for i in range(N_CORES):
        in_maps.append(
            {
                "x": np.ascontiguousarray(np.asarray(x[i], dtype=np.float32).T).astype(bf),
                "W_qkv": wq,
                "b_qkv": bq,
                "W_proj": wp,
                "b_proj": bp,
            }
        )
    res = run_bass_kernel_spmd(
        nc, in_maps, core_ids=list(range(N_CORES)), trace=_trace
    )
    out = np.stack([res.results[i]["out"] for i in range(N_CORES)], axis=0).astype(
        np.float32
    )
    if _trace:
        _CACHE["last_results"] = res
    return out
